# revision 3
# baseline (speedup 1.0000x reference)
"""Trainium2 Bass kernel for DifferentiableLandmarkDetector (top-k soft-argmax).

Full input: heatmap [2, 16, 96, 128, 128] f32.  For each of the 32 (B, C)
slices: top-64 over the flattened 1,572,864-voxel volume, temperature softmax
over the 64 values, probability-weighted (d, h, w) coordinate sum -> [2,16,3].

Strategy (memory-bound regime):
  - Shard the 32 independent (B,C) slices across 8 cores (4 slices = 25.2MB
    per core, contiguous in HBM).
  - Device kernel: stream the shard through SBUF in 1MB tiles on the SP
    HWDGE ring and max-reduce every 64 contiguous voxels (DVE tensor_reduce,
    fp16 out) into SBUF.  This is the single full read of HBM: the 16 DMA
    engines are 99% busy at ~26GB/s each for the whole stream (416GB/s,
    measured) -- the stream is the hard roofline at ~61us/core.
  - Host epilogue (O(100KB) of data): at most 64 groups can contain a top-64
    element (each such group's max >= the 64th largest value), so the top
    groups by group-max provably contain the entire top-64 set; TOP_GROUPS
    256 absorbs fp16 rounding of the group maxes.  Gather those 256*64
    candidates from the input, exact top-64 (jax.lax.top_k tie semantics),
    softmax + coordinate decode in numpy.

Perf notes (measured on HW via NTFF traces; exec window = first MEMSET ->
last COMPARE_BRANCH, so ~5.8us of engine init is uncounted but the ~7.6us
fixed walrus semaphore-file teardown IS counted):
  - Input stream is gapless and engine-limited; tile size/queue changes
    cannot help.  All remaining wins are at the edges.
  - gm writes go on the scalar-engine ring (out-DMAs on the SP ring stall
    input loads) and are CHUNKED: each chunk lands in its own SBUF tile
    (no WAR hazard against later reduces) and its write fires mid-stream
    right after its last reduce, so only a ~1.5KB write trails the stream.
  - Tail tiles taper (1792/1280/640/256/128 cols) so the final DVE reduce
    on the critical path is ~0.2us instead of ~1.2us.
  - Walrus allows only 1 sync-wait per DMA/compute instruction; building via
    bacc.Bacc (generate_event_semaphores splits waits) is required.
"""

import sys

import numpy as np

if "/opt/trn_rl_repo" not in sys.path:
    sys.path.insert(0, "/opt/trn_rl_repo")

TEMPERATURE = 0.1
TOPK = 64
B, C, D, H, W = 2, 16, 96, 128, 128
VOX = D * H * W                          # 1,572,864 voxels per (B,C) slice
N_CORES = 8
SLICES_PER_CORE = (B * C) // N_CORES     # 4
CORE_ELEMS = SLICES_PER_CORE * VOX       # 6,291,456
P = 128                                  # SBUF partitions
GROUP = 64                               # contiguous voxels per group-max
GROUPS_PER_SLICE = VOX // GROUP          # 24,576
N_GROUPS = CORE_ELEMS // GROUP           # 98,304 per core
TOP_GROUPS = 256                         # >= 64 + fp16-rounding slack

# Free-dim widths of the streamed tiles.  Uniform 1MB (2048-col) tiles keep
# the HWDGE descriptor stream at line rate; the taper only shortens the
# final reduce+write on the post-stream critical path (descriptor issue for
# the small tiles happens ~15us ahead of their data, so no stream gap).
TILE_WIDTHS = [2048] * 22 + [1792, 1280, 640, 256, 128]
assert sum(TILE_WIDTHS) * P == CORE_ELEMS
GM_COLS = N_GROUPS // P                  # 768

# gm write chunks: [start_tile, end_tile) -> one SBUF tile + one scalar-ring
# DMA issued right after tile end_tile-1's reduce.  The last chunk is tiny
# (6 cols = 1.5KB) so the post-stream write is as short as possible.
CHUNKS = [(0, 10), (10, 20), (20, 25), (25, 27)]
assert CHUNKS[-1][1] == len(TILE_WIDTHS)

# Set by a caller (e.g. test harness) to profile; LAST_RESULTS then holds the
# BassKernelResults with exec_time_ns.
PROFILE = False
LAST_RESULTS = None

_nc_cache = None


def _build_nc():
    global _nc_cache
    if _nc_cache is not None:
        return _nc_cache
    from concourse import bacc, mybir
    from concourse.tile import TileContext

    nc = bacc.Bacc()
    x = nc.declare_dram_parameter(
        "x", [CORE_ELEMS], mybir.dt.float32, isOutput=False
    )
    gm = nc.declare_dram_parameter(
        "gm", [P, GM_COLS], mybir.dt.float16, isOutput=True
    )

    chunk_cols = [
        sum(w // GROUP for w in TILE_WIDTHS[a:b]) for a, b in CHUNKS
    ]
    with TileContext(nc) as tc:
        with (
            tc.tile_pool(name="data", bufs=10) as pool,
            tc.tile_pool(name="gmp", bufs=1) as gpool,
        ):
            ctiles = [
                gpool.tile([P, cc], mybir.dt.float16, name=f"gmchunk{i}")
                for i, cc in enumerate(chunk_cols)
            ]
            eoff = 0   # element offset into x
            gcol = 0   # global column offset into gm
            ci = 0     # current chunk
            coff = 0   # column offset within current chunk tile
            for ti, w in enumerate(TILE_WIDTHS):
                gw = w // GROUP
                tl = pool.tile([P, w], mybir.dt.float32, tag="data")
                src = x[eoff:eoff + P * w].rearrange("(p f) -> p f", p=P)
                nc.sync.dma_start(out=tl[:], in_=src)
                nc.vector.tensor_reduce(
                    out=ctiles[ci][:, coff:coff + gw],
                    in_=tl[:].rearrange("p (g e) -> p g e", e=GROUP),
                    axis=mybir.AxisListType.X,
                    op=mybir.AluOpType.max,
                )
                eoff += P * w
                gcol += gw
                coff += gw
                if ti == CHUNKS[ci][1] - 1:
                    nc.scalar.dma_start(
                        out=gm[:, gcol - chunk_cols[ci]:gcol],
                        in_=ctiles[ci][:],
                    )
                    ci += 1
                    coff = 0
    nc.finalize()
    _nc_cache = nc
    return nc


def kernel(heatmap) -> np.ndarray:
    global LAST_RESULTS
    from concourse.bass_utils import run_bass_kernel_spmd

    x = np.ascontiguousarray(np.asarray(heatmap), dtype=np.float32)
    assert x.shape == (B, C, D, H, W)
    x2 = x.reshape(B * C, VOX)

    nc = _build_nc()
    in_maps = [
        {"x": np.ascontiguousarray(
            x2[i * SLICES_PER_CORE:(i + 1) * SLICES_PER_CORE].reshape(-1))}
        for i in range(N_CORES)
    ]
    try:
        res = run_bass_kernel_spmd(
            nc, in_maps, list(range(N_CORES)), trace=PROFILE
        )
    except Exception:
        # one retry for transient device/runtime hiccups
        res = run_bass_kernel_spmd(
            nc, in_maps, list(range(N_CORES)), trace=PROFILE
        )
    LAST_RESULTS = res

    ecols = np.arange(GROUP)
    out = np.zeros((B * C, 3), dtype=np.float32)
    for core in range(N_CORES):
        # gm[p, cbase+q] holds the max of core-flat elems
        # [e0 + p*w + 64q, +64), i.e. core-flat group e0/64 + p*(w/64) + q,
        # for the segment starting at element offset e0 / column cbase.
        G2 = res.results[core]["gm"]  # [128, 768] fp16
        Gf = np.empty(N_GROUPS, dtype=np.float16)
        goff = cbase = 0
        for w in TILE_WIDTHS:
            gw = w // GROUP
            Gf[goff:goff + P * gw] = G2[:, cbase:cbase + gw].reshape(-1)
            goff += P * gw
            cbase += gw
        for s in range(SLICES_PER_CORE):
            bc = core * SLICES_PER_CORE + s
            gs = Gf[s * GROUPS_PER_SLICE:(s + 1) * GROUPS_PER_SLICE]
            top_g = np.argpartition(gs, -TOP_GROUPS)[-TOP_GROUPS:]
            fpos = (top_g[:, None] * GROUP + ecols[None, :]).reshape(-1)
            vals = x2[bc, fpos]
            # descending by value, ties -> lower index (jax.lax.top_k order)
            order = np.lexsort((fpos, -vals))[:TOPK]
            v64 = vals[order].astype(np.float64)
            p64 = fpos[order]
            w = v64 / TEMPERATURE
            w -= w.max()
            ew = np.exp(w)
            probs = ew / (ew.sum() + 1e-20)
            d = p64 // (H * W)
            h = (p64 % (H * W)) // W
            wv = p64 % W
            out[bc, 0] = (probs * d).sum()
            out[bc, 1] = (probs * h).sum()
            out[bc, 2] = (probs * wv).sum()
    return out.reshape(B, C, 3)
